# revision 1
# baseline (speedup 1.0000x reference)
"""Causal transformer layer (B=4,T=2048,D=1024,F=4096,H=16) on 8 trn2 NeuronCores.

L1 (attention): core = (batch b, head-group g of 8 heads).
L2 (MLP):       core = contiguous 1024-token chunk of the flattened (B,T).

All matmuls fp32 by default; set MM_DT = mybir.dt.float32r for fast mode
(operand APs are bitcast at the matmul call sites).
"""

import sys

sys.path.insert(0, "/opt/trn_rl_repo")

import numpy as np

import concourse.bass as bass
import concourse.tile as tile
from concourse import bacc, mybir
from concourse.bass_utils import run_bass_kernel_spmd
from concourse.masks import make_identity

F32 = mybir.dt.float32
MM_DT = mybir.dt.float32  # knob: mybir.dt.float32r for 4x matmul rate

B, T, D, F, H, HD = 4, 2048, 1024, 4096, 16, 64
EPS = 1e-6
NT = T // 128          # 16 token tiles (L1)
KD = D // 128          # 8 k-tiles over D
SCALE = HD ** -0.5     # 0.125, folded into exp()


def _mm(ap):
    return ap.bitcast(MM_DT) if MM_DT != F32 else ap


def _bcast(ap, p=128):
    """Broadcast a [N] DRAM vector across p partitions -> [p, N] DMA source."""
    return bass.AP(tensor=ap.tensor, offset=ap.offset, ap=[[0, p], *list(ap.ap)])


def mm(nc, out, lhsT, rhs, **kw):
    nc.tensor.matmul(out, _mm(lhsT), _mm(rhs), **kw)


# --------------------------------------------------------------------------
# L1: attention partial.  Inputs (per core):
#   x        [2048, 1024]  token-major batch slice
#   wqkv     [1024, 1536]  host-packed cols: [q(512) | k(512) | v(512)]
#   wo       [512, 1024]   rows of w_o for this head group
#   cos_rep  [128, 2048]   cos[t, :32].T replicated to 4 row-blocks of 32
#   sin_srep [128, 2048]   sin.T with signs [-,+,-,+] per 32-row block
#   ln1s, ln1b, gattn [1024]
#   tri      [128, 128]    lower-triangular ones
# Output: y [2048, 1024] = (attn_partial @ wo) * gamma_attn  (token-major)
# --------------------------------------------------------------------------
ABLATE = set()


def build_l1():
    nc = bacc.Bacc("TRN2", target_bir_lowering=False, debug=False, num_devices=8)
    x = nc.dram_tensor("x", [T, D], F32, kind="ExternalInput").ap()
    wqkv = nc.dram_tensor("wqkv", [D, 1536], F32, kind="ExternalInput").ap()
    wo = nc.dram_tensor("wo", [512, D], F32, kind="ExternalInput").ap()
    cos_rep = nc.dram_tensor("cos_rep", [128, T], F32, kind="ExternalInput").ap()
    sin_srep = nc.dram_tensor("sin_srep", [128, T], F32, kind="ExternalInput").ap()
    ln1s = nc.dram_tensor("ln1s", [D], F32, kind="ExternalInput").ap()
    ln1b = nc.dram_tensor("ln1b", [D], F32, kind="ExternalInput").ap()
    gattn = nc.dram_tensor("gattn", [D], F32, kind="ExternalInput").ap()
    tri = nc.dram_tensor("tri", [128, 128], F32, kind="ExternalInput").ap()
    y = nc.dram_tensor("y", [T, D], F32, kind="ExternalOutput").ap()

    wqkv_r = wqkv.rearrange("(k p) n -> p k n", p=128)   # [128, 8, 1536]
    wo_r = wo.rearrange("(k p) n -> p k n", p=128)       # [128, 4, 1024]

    with tile.TileContext(nc) as tc:
        with (
            tc.tile_pool(name="singles", bufs=1) as singles,
            tc.tile_pool(name="xnT", bufs=1) as xnTp,
            tc.tile_pool(name="attT", bufs=1) as attTp,
        ):
            ident = singles.tile([128, 128], F32)
            make_identity(nc, ident)
            eps_t = singles.tile([128, 1], F32)
            nc.vector.memset(eps_t, EPS)
            tri_t = singles.tile([128, 128], F32)
            nc.sync.dma_start(out=tri_t, in_=tri)
            cos_t = singles.tile([128, T], F32)
            nc.sync.dma_start(out=cos_t, in_=cos_rep)
            sin_t = singles.tile([128, T], F32)
            nc.sync.dma_start(out=sin_t, in_=sin_srep)
            ln1s_b = singles.tile([128, D], F32)
            nc.sync.dma_start(out=ln1s_b, in_=_bcast(ln1s))
            ln1b_b = singles.tile([128, D], F32)
            nc.sync.dma_start(out=ln1b_b, in_=_bcast(ln1b))
            gattn_b = singles.tile([128, D], F32)
            nc.sync.dma_start(out=gattn_b, in_=_bcast(gattn))

            xnT = [xnTp.tile([128, T], F32, tag=f"xnT{k}", name=f"xnT{k}") for k in range(KD)]
            attT = [attTp.tile([128, T], F32, tag=f"attT{p}", name=f"attT{p}") for p in range(4)]

            # ---- Phase A: LN1 + transpose to feature-major ----
            with (
                tc.tile_pool(name="lnw", bufs=3) as lnw,
                tc.tile_pool(name="lnst", bufs=4) as lnst,
                tc.tile_pool(name="ptr", bufs=4, space="PSUM") as ptr,
            ):
                for tt in range(NT):
                    x_t = lnw.tile([128, D], F32, tag="x_t")
                    nc.sync.dma_start(out=x_t, in_=x[tt * 128:(tt + 1) * 128, :])
                    st = lnst.tile([128, 2, 6], F32, tag="st")
                    for s in range(2):
                        nc.vector.bn_stats(out=st[:, s, :], in_=x_t[:, s * 512:(s + 1) * 512])
                    mv = lnst.tile([128, 2], F32, tag="mv")
                    nc.vector.bn_aggr(out=mv, in_=st)
                    rstd = lnst.tile([128, 1], F32, tag="rstd")
                    nc.scalar.activation(out=rstd, in_=mv[:, 1:2],
                                         func=mybir.ActivationFunctionType.Sqrt,
                                         bias=eps_t)
                    nc.vector.reciprocal(out=rstd, in_=rstd)
                    xn_t = lnw.tile([128, D], F32, tag="xn_t")
                    nc.vector.tensor_scalar(out=xn_t, in0=x_t,
                                            scalar1=mv[:, 0:1], scalar2=rstd,
                                            op0=mybir.AluOpType.subtract,
                                            op1=mybir.AluOpType.mult)
                    nc.vector.tensor_mul(out=xn_t, in0=xn_t, in1=ln1s_b)
                    nc.vector.tensor_add(out=xn_t, in0=xn_t, in1=ln1b_b)
                    for k in range(KD):
                        pt = ptr.tile([128, 128], F32, tag="pt")
                        nc.tensor.transpose(pt, xn_t[:, k * 128:(k + 1) * 128], ident)
                        eng = nc.vector if k % 2 == 0 else nc.scalar
                        if eng is nc.vector:
                            nc.vector.tensor_copy(out=xnT[k][:, tt * 128:(tt + 1) * 128], in_=pt)
                        else:
                            nc.scalar.copy(out=xnT[k][:, tt * 128:(tt + 1) * 128], in_=pt)

            # ---- Phase B: per head-pair projections + attention ----
            for hp in range(4):
                with (
                    tc.tile_pool(name=f"w{hp}", bufs=1) as wp,
                    tc.tile_pool(name=f"qk{hp}", bufs=1) as qkp,
                    tc.tile_pool(name=f"rot{hp}", bufs=2) as rotp,
                    tc.tile_pool(name=f"v{hp}", bufs=1) as vp,
                    tc.tile_pool(name=f"att{hp}", bufs=1) as attp,
                    tc.tile_pool(name=f"ew{hp}", bufs=1) as ewp,
                    tc.tile_pool(name=f"den{hp}", bufs=8) as denp,
                ):
                    wq = wp.tile([128, KD, 128], F32, tag="wq")
                    nc.sync.dma_start(out=wq, in_=wqkv_r[:, :, hp * 128:(hp + 1) * 128])
                    wk = wp.tile([128, KD, 128], F32, tag="wk")
                    nc.sync.dma_start(out=wk, in_=wqkv_r[:, :, 512 + hp * 128:512 + (hp + 1) * 128])
                    wv = wp.tile([128, KD, 128], F32, tag="wv")
                    nc.sync.dma_start(out=wv, in_=wqkv_r[:, :, 1024 + hp * 128:1024 + (hp + 1) * 128])

                    qfin = qkp.tile([128, T], F32, tag="qfin")
                    kfin = qkp.tile([128, T], F32, tag="kfin")
                    vt = [vp.tile([128, 128], F32, tag=f"v{t}", name=f"v{hp}_{t}") for t in range(NT)]
                    with tc.tile_pool(name=f"pqk{hp}", bufs=2, space="PSUM") as pqk:
                        for c in range(4):
                            cs = slice(c * 512, (c + 1) * 512)
                            for nm, wt, dst in (("q", wq, qfin), ("k", wk, kfin)):
                                pp = pqk.tile([128, 512], F32, tag="pp")
                                for k in range(KD):
                                    mm(nc, pp, wt[:, k, :], xnT[k][:, cs],
                                       start=(k == 0), stop=(k == KD - 1))
                                nc.scalar.copy(out=dst[:, cs], in_=pp)
                                if "norope" not in ABLATE:
                                    rt = rotp.tile([128, 512], F32, tag="rt")
                                    for blk in range(4):
                                        src = slice(blk * 32 + (32 if blk % 2 == 0 else -32),
                                                    blk * 32 + (64 if blk % 2 == 0 else 0))
                                        nc.sync.dma_start(out=rt[blk * 32:(blk + 1) * 32, :],
                                                          in_=dst[src, cs])
                                    nc.vector.tensor_mul(out=rt, in0=rt, in1=sin_t[:, cs])
                                    nc.vector.tensor_mul(out=dst[:, cs], in0=dst[:, cs], in1=cos_t[:, cs])
                                    nc.vector.tensor_add(out=dst[:, cs], in0=dst[:, cs], in1=rt)

                        for tt in range(NT):
                            pv = pqk.tile([128, 128], F32, tag="pv")
                            for k in range(KD):
                                mm(nc, pv, xnT[k][:, tt * 128:(tt + 1) * 128], wv[:, k, :],
                                   start=(k == 0), stop=(k == KD - 1))
                            nc.vector.tensor_copy(out=vt[tt], in_=pv)

                    att = [attp.tile([128, 128], F32, tag=f"a{t}", name=f"a{hp}_{t}") for t in range(NT)]
                    with (
                        tc.tile_pool(name=f"ps{hp}", bufs=1, space="PSUM") as psp,
                        tc.tile_pool(name=f"pet{hp}", bufs=2, space="PSUM") as pet,
                        tc.tile_pool(name=f"pav{hp}", bufs=2, space="PSUM") as pav,
                    ):
                        for h in range(2):
                            rb = 64 * h
                            for qt in range(NT):
                                span = (qt + 1) * 128
                                e_t = ewp.tile([128, 2048], F32, tag="e")
                                den = denp.tile([128, 1], F32, tag="den")
                                dwork = denp.tile([128, 1], F32, tag="dwork")
                                if "noscore" not in ABLATE:
                                    ps = psp.tile([128, 2048], F32, tag="ps")
                                    nchunk = (span + 511) // 512
                                    for j in range(nchunk):
                                        n0, n1 = j * 512, min((j + 1) * 512, span)
                                        mm(nc, ps[:, n0:n1],
                                           qfin[rb:rb + 64, qt * 128:(qt + 1) * 128],
                                           kfin[rb:rb + 64, n0:n1])
                                    if qt > 0:
                                        nc.scalar.activation(out=e_t[:, :qt * 128], in_=ps[:, :qt * 128],
                                                             func=mybir.ActivationFunctionType.Exp,
                                                             scale=SCALE, accum_out=den)
                                    nc.scalar.activation(out=e_t[:, qt * 128:span],
                                                         in_=ps[:, qt * 128:span],
                                                         func=mybir.ActivationFunctionType.Exp,
                                                         scale=SCALE)
                                else:
                                    nc.vector.memset(e_t[:, :span], 0.001)
                                nc.vector.tensor_mul(out=e_t[:, qt * 128:span],
                                                     in0=e_t[:, qt * 128:span], in1=tri_t)
                                nc.vector.reduce_sum(out=dwork, in_=e_t[:, qt * 128:span],
                                                     axis=mybir.AxisListType.X)
                                if qt > 0 and "noscore" not in ABLATE:
                                    nc.vector.tensor_add(out=den, in0=den, in1=dwork)
                                else:
                                    nc.vector.tensor_copy(out=den, in_=dwork)
                                nc.vector.reciprocal(out=den, in_=den)
                                if "noav" not in ABLATE:
                                    pa = pav.tile([128, 64], F32, tag="pa")
                                    for kt in range(qt + 1):
                                        pe_t = pet.tile([128, 128], F32, tag="pet")
                                        nc.tensor.transpose(pe_t, e_t[:, kt * 128:(kt + 1) * 128], ident)
                                        et = ewp.tile([128, 128], F32, tag="et")
                                        if kt % 2 == 0:
                                            nc.vector.tensor_copy(out=et, in_=pe_t)
                                        else:
                                            nc.scalar.copy(out=et, in_=pe_t)
                                        mm(nc, pa, et, vt[kt][:, rb:rb + 64],
                                           start=(kt == 0), stop=(kt == qt))
                                    nc.vector.tensor_scalar_mul(out=att[qt][:, rb:rb + 64],
                                                                in0=pa, scalar1=den)
                                else:
                                    nc.vector.tensor_scalar_mul(out=att[qt][:, rb:rb + 64],
                                                                in0=e_t[:, :64], scalar1=den)
                        for tt in range(NT):
                            pt2 = pet.tile([128, 128], F32, tag="pet")
                            nc.tensor.transpose(pt2, att[tt], ident)
                            nc.scalar.copy(out=attT[hp][:, tt * 128:(tt + 1) * 128], in_=pt2)

            # ---- Phase C: O projection + gamma_attn ----
            with (
                tc.tile_pool(name="wop", bufs=1) as wop,
                tc.tile_pool(name="yw", bufs=3) as yw,
                tc.tile_pool(name="po", bufs=2, space="PSUM") as pop,
            ):
                wo_t = wop.tile([128, 4, D], F32)
                nc.sync.dma_start(out=wo_t, in_=wo_r)
                for tt in range(NT):
                    y_t = yw.tile([128, D], F32, tag="y_t")
                    for dc in range(2):
                        po = pop.tile([128, 512], F32, tag="po")
                        for hp in range(4):
                            mm(nc, po, attT[hp][:, tt * 128:(tt + 1) * 128],
                               wo_t[:, hp, dc * 512:(dc + 1) * 512],
                               start=(hp == 0), stop=(hp == 3))
                        nc.vector.tensor_mul(out=y_t[:, dc * 512:(dc + 1) * 512],
                                             in0=po, in1=gattn_b[:, dc * 512:(dc + 1) * 512])
                    nc.sync.dma_start(out=y[tt * 128:(tt + 1) * 128, :], in_=y_t)

    nc.compile()
    return nc


# --------------------------------------------------------------------------
# L2: MLP.  Inputs (per core, 1024-token chunk):
#   xc, ya, yb [1024, 1024] token-major; x2 = xc + ya + yb
#   ln2s, ln2b, gmlp [1024]
#   wg, wu [1024, 4096], wd [4096, 1024]
# Output: out [1024, 1024] = x2 + gmlp * (gelu_tanh(xn2@wg) * (xn2@wu)) @ wd
# --------------------------------------------------------------------------
def build_l2():
    nc = bacc.Bacc("TRN2", target_bir_lowering=False, debug=False, num_devices=8)
    TC = 1024
    xc = nc.dram_tensor("xc", [TC, D], F32, kind="ExternalInput").ap()
    ya = nc.dram_tensor("ya", [TC, D], F32, kind="ExternalInput").ap()
    yb = nc.dram_tensor("yb", [TC, D], F32, kind="ExternalInput").ap()
    ln2s = nc.dram_tensor("ln2s", [D], F32, kind="ExternalInput").ap()
    ln2b = nc.dram_tensor("ln2b", [D], F32, kind="ExternalInput").ap()
    gmlp = nc.dram_tensor("gmlp", [D], F32, kind="ExternalInput").ap()
    wg = nc.dram_tensor("wg", [D, F], F32, kind="ExternalInput").ap()
    wu = nc.dram_tensor("wu", [D, F], F32, kind="ExternalInput").ap()
    wd = nc.dram_tensor("wd", [F, D], F32, kind="ExternalInput").ap()
    out = nc.dram_tensor("out", [TC, D], F32, kind="ExternalOutput").ap()

    wg_r = wg.rearrange("(k p) n -> p k n", p=128)   # [128, 8, 4096]
    wu_r = wu.rearrange("(k p) n -> p k n", p=128)
    wd_r = wd.rearrange("(a p) n -> a p n", p=128)   # [32, 128, 1024]
    NTC = TC // 128  # 8

    with tile.TileContext(nc) as tc:
        with (
            tc.tile_pool(name="singles", bufs=1) as singles,
            tc.tile_pool(name="x2p", bufs=1) as x2p,
            tc.tile_pool(name="xnTp", bufs=1) as xnTp,
            tc.tile_pool(name="dac", bufs=1) as dacp,
        ):
            ident = singles.tile([128, 128], F32)
            make_identity(nc, ident)
            eps_t = singles.tile([128, 1], F32)
            nc.vector.memset(eps_t, EPS)
            ln2s_b = singles.tile([128, D], F32)
            nc.sync.dma_start(out=ln2s_b, in_=_bcast(ln2s))
            ln2b_b = singles.tile([128, D], F32)
            nc.sync.dma_start(out=ln2b_b, in_=_bcast(ln2b))
            gmlp_b = singles.tile([128, D], F32)
            nc.sync.dma_start(out=gmlp_b, in_=_bcast(gmlp))

            x2 = [x2p.tile([128, D], F32, tag=f"x2{t}", name=f"x2_{t}") for t in range(NTC)]
            xn2T = [xnTp.tile([128, TC], F32, tag=f"xn2T{k}", name=f"xn2T{k}") for k in range(KD)]
            daccT = [dacp.tile([128, TC], F32, tag=f"dac{d}", name=f"dac{d}") for d in range(KD)]

            # ---- Phase 1: residual add + LN2 + transpose ----
            with (
                tc.tile_pool(name="lnw", bufs=3) as lnw,
                tc.tile_pool(name="lnst", bufs=4) as lnst,
                tc.tile_pool(name="ptr", bufs=4, space="PSUM") as ptr,
            ):
                for tt in range(NTC):
                    rs = slice(tt * 128, (tt + 1) * 128)
                    a_t = lnw.tile([128, D], F32, tag="a_t")
                    nc.sync.dma_start(out=a_t, in_=ya[rs, :])
                    b_t = lnw.tile([128, D], F32, tag="b_t")
                    nc.sync.dma_start(out=b_t, in_=yb[rs, :])
                    c_t = lnw.tile([128, D], F32, tag="c_t")
                    nc.sync.dma_start(out=c_t, in_=xc[rs, :])
                    nc.vector.tensor_add(out=a_t, in0=a_t, in1=b_t)
                    nc.vector.tensor_add(out=x2[tt], in0=a_t, in1=c_t)
                    st = lnst.tile([128, 2, 6], F32, tag="st")
                    for s in range(2):
                        nc.vector.bn_stats(out=st[:, s, :], in_=x2[tt][:, s * 512:(s + 1) * 512])
                    mv = lnst.tile([128, 2], F32, tag="mv")
                    nc.vector.bn_aggr(out=mv, in_=st)
                    rstd = lnst.tile([128, 1], F32, tag="rstd")
                    nc.scalar.activation(out=rstd, in_=mv[:, 1:2],
                                         func=mybir.ActivationFunctionType.Sqrt,
                                         bias=eps_t)
                    nc.vector.reciprocal(out=rstd, in_=rstd)
                    xn_t = lnw.tile([128, D], F32, tag="xn_t")
                    nc.vector.tensor_scalar(out=xn_t, in0=x2[tt],
                                            scalar1=mv[:, 0:1], scalar2=rstd,
                                            op0=mybir.AluOpType.subtract,
                                            op1=mybir.AluOpType.mult)
                    nc.vector.tensor_mul(out=xn_t, in0=xn_t, in1=ln2s_b)
                    nc.vector.tensor_add(out=xn_t, in0=xn_t, in1=ln2b_b)
                    for k in range(KD):
                        pt = ptr.tile([128, 128], F32, tag="pt")
                        nc.tensor.transpose(pt, xn_t[:, k * 128:(k + 1) * 128], ident)
                        if k % 2 == 0:
                            nc.vector.tensor_copy(out=xn2T[k][:, tt * 128:(tt + 1) * 128], in_=pt)
                        else:
                            nc.scalar.copy(out=xn2T[k][:, tt * 128:(tt + 1) * 128], in_=pt)

            # ---- Phase 2: gate/up/gelu/down, F in groups of 4x128 ----
            with (
                tc.tile_pool(name="wgu", bufs=4) as wgup,
                tc.tile_pool(name="wdp", bufs=6) as wdp,
                tc.tile_pool(name="mp", bufs=5) as mp,
                tc.tile_pool(name="pg", bufs=2, space="PSUM") as pgp,
                tc.tile_pool(name="pu", bufs=2, space="PSUM") as pup,
                tc.tile_pool(name="pd", bufs=2, space="PSUM") as pdp,
            ):
                for fg in range(8):
                    m_ts = []
                    for fi in range(4):
                        f = fg * 4 + fi
                        fs = slice(f * 128, (f + 1) * 128)
                        wg_t = wgup.tile([128, KD, 128], F32, tag="wg")
                        nc.sync.dma_start(out=wg_t, in_=wg_r[:, :, fs])
                        wu_t = wgup.tile([128, KD, 128], F32, tag="wu")
                        nc.sync.dma_start(out=wu_t, in_=wu_r[:, :, fs])
                        m_t = mp.tile([128, TC], F32, tag="m")
                        for c in range(2):
                            cs = slice(c * 512, (c + 1) * 512)
                            pgt = pgp.tile([128, 512], F32, tag="pg")
                            put = pup.tile([128, 512], F32, tag="pu")
                            for k in range(KD):
                                mm(nc, pgt, wg_t[:, k, :], xn2T[k][:, cs],
                                   start=(k == 0), stop=(k == KD - 1))
                            for k in range(KD):
                                mm(nc, put, wu_t[:, k, :], xn2T[k][:, cs],
                                   start=(k == 0), stop=(k == KD - 1))
                            nc.scalar.activation(out=m_t[:, cs], in_=pgt,
                                                 func=mybir.ActivationFunctionType.Gelu_apprx_tanh)
                            nc.vector.tensor_mul(out=m_t[:, cs], in0=m_t[:, cs], in1=put)
                        m_ts.append(m_t)
                    wd_ts = []
                    for fi in range(4):
                        wd_t = wdp.tile([128, D], F32, tag="wd")
                        nc.sync.dma_start(out=wd_t, in_=wd_r[fg * 4 + fi])
                        wd_ts.append(wd_t)
                    for dout in range(KD):
                        ds_ = slice(dout * 128, (dout + 1) * 128)
                        for c in range(2):
                            cs = slice(c * 512, (c + 1) * 512)
                            pdt = pdp.tile([128, 512], F32, tag="pd")
                            for fi in range(4):
                                mm(nc, pdt, wd_ts[fi][:, ds_], m_ts[fi][:, cs],
                                   start=(fi == 0), stop=(fi == 3))
                            if fg == 0:
                                nc.vector.tensor_copy(out=daccT[dout][:, cs], in_=pdt)
                            else:
                                nc.vector.tensor_add(out=daccT[dout][:, cs],
                                                     in0=daccT[dout][:, cs], in1=pdt)

            # ---- Phase 3: transpose back + residual ----
            with (
                tc.tile_pool(name="ow", bufs=3) as ow,
                tc.tile_pool(name="ptr2", bufs=4, space="PSUM") as ptr2,
            ):
                for tt in range(NTC):
                    o_t = ow.tile([128, D], F32, tag="o_t")
                    for dout in range(KD):
                        pt = ptr2.tile([128, 128], F32, tag="pt")
                        nc.tensor.transpose(pt, daccT[dout][:, tt * 128:(tt + 1) * 128], ident)
                        if dout % 2 == 0:
                            nc.vector.tensor_copy(out=o_t[:, dout * 128:(dout + 1) * 128], in_=pt)
                        else:
                            nc.scalar.copy(out=o_t[:, dout * 128:(dout + 1) * 128], in_=pt)
                    nc.vector.tensor_mul(out=o_t, in0=o_t, in1=gmlp_b)
                    nc.vector.tensor_add(out=o_t, in0=o_t, in1=x2[tt])
                    nc.sync.dma_start(out=out[tt * 128:(tt + 1) * 128, :], in_=o_t)

    nc.compile()
    return nc


# --------------------------------------------------------------------------
# Host orchestration
# --------------------------------------------------------------------------
def prep_l1_inputs(x, cos, sin, ln1_scale, ln1_bias, w_qkv, w_o, gamma_attn):
    cosT = np.ascontiguousarray(cos.T)          # [32, 2048]
    sinT = np.ascontiguousarray(sin.T)
    cos_rep = np.tile(cosT, (4, 1)).astype(np.float32)              # [128, 2048]
    sin_srep = np.concatenate([-sinT, sinT, -sinT, sinT], 0).astype(np.float32)
    tri = np.tril(np.ones((128, 128), np.float32))
    wq, wk, wv = w_qkv[:, :D], w_qkv[:, D:2 * D], w_qkv[:, 2 * D:]
    maps = []
    for core in range(8):
        b, g = core // 2, core % 2
        cols = slice(g * 512, (g + 1) * 512)
        wqkv_c = np.concatenate([wq[:, cols], wk[:, cols], wv[:, cols]], 1)
        maps.append({
            "x": np.ascontiguousarray(x[b]),
            "wqkv": np.ascontiguousarray(wqkv_c),
            "wo": np.ascontiguousarray(w_o[cols, :]),
            "cos_rep": cos_rep, "sin_srep": sin_srep,
            "ln1s": ln1_scale, "ln1b": ln1_bias, "gattn": gamma_attn,
            "tri": tri,
        })
    return maps


def prep_l2_inputs(x, y_cores, ln2_scale, ln2_bias, w_gate, w_up, w_down, gamma_mlp):
    maps = []
    for core in range(8):
        b, half = core // 2, core % 2
        rs = slice(half * 1024, (half + 1) * 1024)
        maps.append({
            "xc": np.ascontiguousarray(x[b][rs]),
            "ya": np.ascontiguousarray(y_cores[2 * b][rs]),
            "yb": np.ascontiguousarray(y_cores[2 * b + 1][rs]),
            "ln2s": ln2_scale, "ln2b": ln2_bias, "gmlp": gamma_mlp,
            "wg": w_gate, "wu": w_up, "wd": w_down,
        })
    return maps


_NC_CACHE = {}


def run(x, cos, sin, ln1_scale, ln1_bias, w_qkv, w_o, gamma_attn,
        ln2_scale, ln2_bias, w_gate, w_up, w_down, gamma_mlp,
        trace=False):
    f32 = lambda a: np.asarray(a, np.float32)
    x = f32(x)
    if "l1" not in _NC_CACHE:
        _NC_CACHE["l1"] = build_l1()
    if "l2" not in _NC_CACHE:
        _NC_CACHE["l2"] = build_l2()
    m1 = prep_l1_inputs(x, f32(cos), f32(sin), f32(ln1_scale), f32(ln1_bias),
                        f32(w_qkv), f32(w_o), f32(gamma_attn))
    r1 = run_bass_kernel_spmd(_NC_CACHE["l1"], m1, core_ids=list(range(8)), trace=trace)
    y_cores = [r1.results[i]["y"] for i in range(8)]
    m2 = prep_l2_inputs(x, y_cores, f32(ln2_scale), f32(ln2_bias),
                        f32(w_gate), f32(w_up), f32(w_down), f32(gamma_mlp))
    r2 = run_bass_kernel_spmd(_NC_CACHE["l2"], m2, core_ids=list(range(8)), trace=trace)
    out = np.empty((B, T, D), np.float32)
    for core in range(8):
        b, half = core // 2, core % 2
        out[b, half * 1024:(half + 1) * 1024] = r2.results[core]["out"]
    return out, (r1, r2)


def kernel(x, cos, sin, ln1_scale, ln1_bias, w_qkv, w_o, gamma_attn,
           ln2_scale, ln2_bias, w_gate, w_up, w_down, gamma_mlp):
    """Full-input / full-output entry point. Shards across 8 NeuronCores."""
    out, _ = run(x, cos, sin, ln1_scale, ln1_bias, w_qkv, w_o, gamma_attn,
                 ln2_scale, ln2_bias, w_gate, w_up, w_down, gamma_mlp)
    return out



# revision 6
# speedup vs baseline: 2.0710x; 2.0710x over previous
"""Causal transformer layer (B=4,T=2048,D=1024,F=4096,H=16) on 8 trn2 NeuronCores.

L1 (attention): core = (batch b, head-group g of 8 heads).
L2 (MLP):       core = contiguous 1024-token chunk of the flattened (B,T).

All matmul operands bf16 (1 cycle/row on TRN2 PE vs 4 for fp32), fp32 PSUM
accumulation. Attention computes scores TRANSPOSED (sT[k,q]) so exp output
feeds the A@V matmul directly as the moving operand — no per-tile PE
transposes of the attention matrix, and the softmax denominator falls out of
a ones-column folded into V. Normalization is a rank-1 broadcast matmul.
"""

import sys

sys.path.insert(0, "/opt/trn_rl_repo")

import numpy as np
import ml_dtypes

import concourse.bass as bass
import concourse.tile as tile
from concourse import bacc, mybir
from concourse.bass_utils import run_bass_kernel_spmd
from concourse.masks import make_identity

F32 = mybir.dt.float32
BF16 = mybir.dt.bfloat16
NPBF = ml_dtypes.bfloat16

B, T, D, F, H, HD = 4, 2048, 1024, 4096, 16, 64
EPS = 1e-6
NT = T // 128          # 16 token tiles (L1)
KD = D // 128          # 8 k-tiles over D
SCALE = HD ** -0.5     # 0.125, folded into exp()


def _bcast(ap, p=128):
    """Broadcast a [N] DRAM vector across p partitions -> [p, N] DMA source."""
    return bass.AP(tensor=ap.tensor, offset=ap.offset, ap=[[0, p], *list(ap.ap)])


# --------------------------------------------------------------------------
# L1: attention.  Inputs (per core):
#   x        [2048, 1024] f32   token-major batch slice
#   wqkv     [4, 3, 128, 8, 128] bf16  per-hp packed q/k/v weight tiles
#   wo       [128, 4, 1024] bf16
#   cos_rep  [128, 2048] bf16   cos[t, :32].T replicated to 4 row-blocks of 32
#   sin_srep [128, 2048] bf16   sin.T with signs [-,+,-,+] per 32-row block
#   ln1s, ln1b, gattn [1024] f32
#   triu     [128, 128] bf16    upper-triangular ones (k<=q mask in sT layout)
# Output: y [2048, 1024] bf16 = (attn @ wo) * gamma_attn  (token-major)
# --------------------------------------------------------------------------
def build_l1():
    nc = bacc.Bacc("TRN2", target_bir_lowering=False, debug=False, num_devices=8)
    x = nc.dram_tensor("x", [T, D], F32, kind="ExternalInput").ap()
    wqkv = nc.dram_tensor("wqkv", [4, 3, 128, KD, 128], BF16, kind="ExternalInput").ap()
    wo = nc.dram_tensor("wo", [128, 4, D], BF16, kind="ExternalInput").ap()
    cos_rep = nc.dram_tensor("cos_rep", [128, T], BF16, kind="ExternalInput").ap()
    sin_srep = nc.dram_tensor("sin_srep", [128, T], BF16, kind="ExternalInput").ap()
    ln1s = nc.dram_tensor("ln1s", [D], F32, kind="ExternalInput").ap()
    ln1b = nc.dram_tensor("ln1b", [D], F32, kind="ExternalInput").ap()
    gattn = nc.dram_tensor("gattn", [D], F32, kind="ExternalInput").ap()
    triu = nc.dram_tensor("triu", [128, 128], BF16, kind="ExternalInput").ap()
    y = nc.dram_tensor("y", [T, D], BF16, kind="ExternalOutput").ap()

    with tile.TileContext(nc) as tc:
        with (
            tc.tile_pool(name="singles", bufs=1) as singles,
            tc.tile_pool(name="xnTp", bufs=1) as xnTp,
            tc.tile_pool(name="attTp", bufs=1) as attTp,
            tc.tile_pool(name="vtp", bufs=1) as vtp,
            tc.tile_pool(name="wp", bufs=1) as wp,
        ):
            ident = singles.tile([128, 128], BF16)
            make_identity(nc, ident)
            eps_t = singles.tile([128, 1], F32)
            nc.vector.memset(eps_t, EPS)
            triu_t = singles.tile([128, 128], BF16)
            nc.sync.dma_start(out=triu_t, in_=triu)
            cos_t = singles.tile([128, T], BF16)
            nc.sync.dma_start(out=cos_t, in_=cos_rep)
            sin_t = singles.tile([128, T], BF16)
            nc.sync.dma_start(out=sin_t, in_=sin_srep)
            ln1s_b = singles.tile([128, D], F32)
            nc.sync.dma_start(out=ln1s_b, in_=_bcast(ln1s))
            ln1b_b = singles.tile([128, D], F32)
            nc.sync.dma_start(out=ln1b_b, in_=_bcast(ln1b))
            gattn_b = singles.tile([128, D], F32)
            nc.sync.dma_start(out=gattn_b, in_=_bcast(gattn))
            ones64 = singles.tile([1, 64], BF16)
            nc.vector.memset(ones64, 1.0)

            # all projection weights up front (DMA overlaps phase A)
            wq_t = [wp.tile([128, KD, 128], BF16, tag=f"wq{hp}", name=f"wq{hp}")
                    for hp in range(4)]
            wk_t = [wp.tile([128, KD, 128], BF16, tag=f"wk{hp}", name=f"wk{hp}")
                    for hp in range(4)]
            wv_t = [wp.tile([128, KD, 128], BF16, tag=f"wv{hp}", name=f"wv{hp}")
                    for hp in range(4)]
            for hp in range(4):
                nc.sync.dma_start(out=wq_t[hp], in_=wqkv[hp, 0])
                nc.sync.dma_start(out=wk_t[hp], in_=wqkv[hp, 1])
                nc.sync.dma_start(out=wv_t[hp], in_=wqkv[hp, 2])
            wo_t = singles.tile([128, 4, D], BF16)
            nc.sync.dma_start(out=wo_t, in_=wo)

            xnT = [xnTp.tile([128, T], BF16, tag=f"xnT{k}", name=f"xnT{k}")
                   for k in range(KD)]
            attT = [attTp.tile([128, T], BF16, tag=f"attT{p}", name=f"attT{p}")
                    for p in range(4)]
            # v per (hp, token-tile): [128 ktok, 65+65] = [v_h0 | 1 | v_h1 | 1]
            vt = [[vtp.tile([128, 130], BF16, tag=f"vt{hp}_{t}", name=f"vt{hp}_{t}")
                   for t in range(NT)] for hp in range(4)]
            for hp in range(4):
                for t in range(NT):
                    nc.vector.memset(vt[hp][t][:, 64:65], 1.0)
                    nc.vector.memset(vt[hp][t][:, 129:130], 1.0)

            # ---- Phase A: LN1 + transpose to feature-major + V projection ----
            with (
                tc.tile_pool(name="lnw", bufs=3) as lnw,
                tc.tile_pool(name="lnst", bufs=4) as lnst,
                tc.tile_pool(name="ptr", bufs=4, space="PSUM") as ptr,
                tc.tile_pool(name="pvp", bufs=2, space="PSUM") as pvp,
            ):
                for tt in range(NT):
                    ts_ = slice(tt * 128, (tt + 1) * 128)
                    x_t = lnw.tile([128, D], F32, tag="x_t")
                    nc.sync.dma_start(out=x_t, in_=x[ts_, :])
                    st = lnst.tile([128, 2, 6], F32, tag="st")
                    for s in range(2):
                        nc.vector.bn_stats(out=st[:, s, :], in_=x_t[:, s * 512:(s + 1) * 512])
                    mv = lnst.tile([128, 2], F32, tag="mv")
                    nc.vector.bn_aggr(out=mv, in_=st)
                    rstd = lnst.tile([128, 1], F32, tag="rstd")
                    nc.scalar.activation(out=rstd, in_=mv[:, 1:2],
                                         func=mybir.ActivationFunctionType.Sqrt,
                                         bias=eps_t)
                    nc.vector.reciprocal(out=rstd, in_=rstd)
                    xw = lnw.tile([128, D], F32, tag="xw")
                    nc.vector.tensor_scalar(out=xw, in0=x_t,
                                            scalar1=mv[:, 0:1], scalar2=rstd,
                                            op0=mybir.AluOpType.subtract,
                                            op1=mybir.AluOpType.mult)
                    nc.vector.tensor_mul(out=xw, in0=xw, in1=ln1s_b)
                    xn_t = lnw.tile([128, D], BF16, tag="xn_t")
                    nc.vector.tensor_add(out=xn_t, in0=xw, in1=ln1b_b)
                    for k in range(KD):
                        pt = ptr.tile([128, 128], BF16, tag="pt")
                        nc.tensor.transpose(pt, xn_t[:, k * 128:(k + 1) * 128], ident)
                        if k % 2 == 0:
                            nc.vector.tensor_copy(out=xnT[k][:, ts_], in_=pt)
                        else:
                            nc.scalar.copy(out=xnT[k][:, ts_], in_=pt)
                    for hp in range(4):
                        pv = pvp.tile([128, 128], F32, tag="pv")
                        for k in range(KD):
                            nc.tensor.matmul(pv, xnT[k][:, ts_], wv_t[hp][:, k, :],
                                             start=(k == 0), stop=(k == KD - 1))
                        nc.vector.tensor_copy(out=vt[hp][tt][:, 0:64], in_=pv[:, 0:64])
                        nc.vector.tensor_copy(out=vt[hp][tt][:, 65:129], in_=pv[:, 64:128])

            # ---- Phase B: per head-pair Q/K projection + rope + attention ----
            for hp in range(4):
                with (
                    tc.tile_pool(name=f"qk{hp}", bufs=1) as qkp,
                    tc.tile_pool(name=f"rot{hp}", bufs=2) as rotp,
                    tc.tile_pool(name=f"ew{hp}", bufs=3) as ewp,
                    tc.tile_pool(name=f"den{hp}", bufs=4) as denp,
                    tc.tile_pool(name=f"pqk{hp}", bufs=2, space="PSUM") as pqk,
                    tc.tile_pool(name=f"psc{hp}", bufs=2, space="PSUM") as psc,
                    tc.tile_pool(name=f"pat{hp}", bufs=2, space="PSUM") as pat,
                    tc.tile_pool(name=f"prb{hp}", bufs=2, space="PSUM") as prb,
                ):
                    qfin = qkp.tile([128, T], BF16, tag="qfin")
                    kfin = qkp.tile([128, T], BF16, tag="kfin")
                    for c in range(4):
                        cs = slice(c * 512, (c + 1) * 512)
                        for wt, dst in ((wq_t[hp], qfin), (wk_t[hp], kfin)):
                            pp = pqk.tile([128, 512], F32, tag="pp")
                            for k in range(KD):
                                nc.tensor.matmul(pp, wt[:, k, :], xnT[k][:, cs],
                                                 start=(k == 0), stop=(k == KD - 1))
                            nc.vector.tensor_copy(out=dst[:, cs], in_=pp)
                            # rope: rotate-half via 32-row SBUF shuffle DMA
                            rt = rotp.tile([128, 512], BF16, tag="rt")
                            for blk in range(4):
                                src = slice(blk * 32 + (32 if blk % 2 == 0 else -32),
                                            blk * 32 + (64 if blk % 2 == 0 else 0))
                                nc.sync.dma_start(out=rt[blk * 32:(blk + 1) * 32, :],
                                                  in_=dst[src, cs])
                            nc.vector.tensor_mul(out=rt, in0=rt, in1=sin_t[:, cs])
                            nc.vector.tensor_mul(out=dst[:, cs], in0=dst[:, cs],
                                                 in1=cos_t[:, cs])
                            nc.vector.tensor_add(out=dst[:, cs], in0=dst[:, cs], in1=rt)

                    for h in range(2):
                        rb = 64 * h
                        vs = slice(65 * h, 65 * h + 65)
                        for qc in range(4):
                            q0 = qc * 512
                            attp = pat.tile([65, 512], F32, tag="attp")
                            kt_max = 4 * qc + 3
                            for kt in range(kt_max + 1):
                                v0r = max(0, kt * 128 - q0)
                                w = 512 - v0r
                                ps = psc.tile([128, 512], F32, tag="ps")
                                nc.tensor.matmul(
                                    ps[:, :w],
                                    kfin[rb:rb + 64, kt * 128:(kt + 1) * 128],
                                    qfin[rb:rb + 64, q0 + v0r:q0 + 512])
                                eT = ewp.tile([128, 512], BF16, tag="eT")
                                nc.scalar.activation(out=eT[:, :w], in_=ps[:, :w],
                                                     func=mybir.ActivationFunctionType.Exp,
                                                     scale=SCALE)
                                if kt >= 4 * qc:
                                    nc.vector.tensor_mul(out=eT[:, 0:128],
                                                         in0=eT[:, 0:128], in1=triu_t)
                                nc.tensor.matmul(attp[:, v0r:512], vt[hp][kt][:, vs],
                                                 eT[:, :w],
                                                 start=(kt == 0), stop=(kt == kt_max),
                                                 skip_group_check=True)
                            recd = denp.tile([1, 512], F32, tag="recd")
                            nc.vector.reciprocal(out=recd, in_=attp[64:65, :])
                            recb = denp.tile([1, 512], BF16, tag="recb")
                            nc.vector.tensor_copy(out=recb, in_=recd)
                            rbp = prb.tile([64, 512], F32, tag="rbp")
                            nc.tensor.matmul(rbp, ones64, recb)
                            rbs = denp.tile([64, 512], BF16, tag="rbs")
                            nc.scalar.copy(out=rbs, in_=rbp)
                            nc.vector.tensor_mul(
                                out=attT[hp][rb:rb + 64, q0:q0 + 512],
                                in0=attp[0:64, :], in1=rbs)

            # ---- Phase C: O projection + gamma_attn ----
            with (
                tc.tile_pool(name="yw", bufs=3) as yw,
                tc.tile_pool(name="po", bufs=2, space="PSUM") as pop,
            ):
                for tt in range(NT):
                    ts_ = slice(tt * 128, (tt + 1) * 128)
                    y_t = yw.tile([128, D], BF16, tag="y_t")
                    for dc in range(2):
                        po = pop.tile([128, 512], F32, tag="po")
                        for hp in range(4):
                            nc.tensor.matmul(po, attT[hp][:, ts_],
                                             wo_t[:, hp, dc * 512:(dc + 1) * 512],
                                             start=(hp == 0), stop=(hp == 3))
                        nc.vector.tensor_mul(out=y_t[:, dc * 512:(dc + 1) * 512],
                                             in0=po, in1=gattn_b[:, dc * 512:(dc + 1) * 512])
                    nc.sync.dma_start(out=y[ts_, :], in_=y_t)

    nc.compile()
    return nc


# --------------------------------------------------------------------------
# L2: MLP.  Inputs (per core, 1024-token chunk):
#   xc [1024, 1024] f32; ya, yb [1024, 1024] bf16; x2 = xc + ya + yb
#   ln2s, ln2b, gmlp [1024] f32
#   wgu [32, 128, 2, 8, 128] bf16 (per f-tile gate|up packed)
#   wd  [32, 128, 1024] bf16     (f-major row tiles)
# Output: out [1024, 1024] f32 = x2 + gmlp * (gelu_tanh(xn2@wg) * (xn2@wu)) @ wd
# --------------------------------------------------------------------------
def build_l2():
    nc = bacc.Bacc("TRN2", target_bir_lowering=False, debug=False, num_devices=8)
    TC = 1024
    NTC = TC // 128  # 8
    NF = F // 128    # 32
    xc = nc.dram_tensor("xc", [TC, D], F32, kind="ExternalInput").ap()
    ya = nc.dram_tensor("ya", [TC, D], BF16, kind="ExternalInput").ap()
    yb = nc.dram_tensor("yb", [TC, D], BF16, kind="ExternalInput").ap()
    ln2s = nc.dram_tensor("ln2s", [D], F32, kind="ExternalInput").ap()
    ln2b = nc.dram_tensor("ln2b", [D], F32, kind="ExternalInput").ap()
    gmlp = nc.dram_tensor("gmlp", [D], F32, kind="ExternalInput").ap()
    wgu = nc.dram_tensor("wgu", [NF, 128, 2, KD, 128], BF16, kind="ExternalInput").ap()
    wd = nc.dram_tensor("wd", [NF, 128, D], BF16, kind="ExternalInput").ap()
    out = nc.dram_tensor("out", [TC, D], F32, kind="ExternalOutput").ap()

    with tile.TileContext(nc) as tc:
        with (
            tc.tile_pool(name="singles", bufs=1) as singles,
            tc.tile_pool(name="xnTp", bufs=1) as xnTp,
            tc.tile_pool(name="mp", bufs=1) as mpool,
        ):
            ident = singles.tile([128, 128], BF16)
            make_identity(nc, ident)
            eps_t = singles.tile([128, 1], F32)
            nc.vector.memset(eps_t, EPS)
            ln2s_b = singles.tile([128, D], F32)
            nc.sync.dma_start(out=ln2s_b, in_=_bcast(ln2s))
            ln2b_b = singles.tile([128, D], F32)
            nc.sync.dma_start(out=ln2b_b, in_=_bcast(ln2b))
            gmlp_b = singles.tile([128, D], F32)
            nc.sync.dma_start(out=gmlp_b, in_=_bcast(gmlp))

            xn2T = [xnTp.tile([128, TC], BF16, tag=f"xn2T{k}", name=f"xn2T{k}")
                    for k in range(KD)]
            m = [mpool.tile([128, TC], BF16, tag=f"m{f}", name=f"m{f}")
                 for f in range(NF)]

            # ---- Phase 1: residual add + LN2 + transpose ----
            with (
                tc.tile_pool(name="lnw", bufs=2) as lnw,
                tc.tile_pool(name="lnst", bufs=4) as lnst,
                tc.tile_pool(name="ptr", bufs=4, space="PSUM") as ptr,
            ):
                for tt in range(NTC):
                    ts_ = slice(tt * 128, (tt + 1) * 128)
                    a_t = lnw.tile([128, D], BF16, tag="a_t")
                    nc.sync.dma_start(out=a_t, in_=ya[ts_, :])
                    b_t = lnw.tile([128, D], BF16, tag="b_t")
                    nc.sync.dma_start(out=b_t, in_=yb[ts_, :])
                    c_t = lnw.tile([128, D], F32, tag="c_t")
                    nc.sync.dma_start(out=c_t, in_=xc[ts_, :])
                    s_t = lnw.tile([128, D], F32, tag="s_t")
                    nc.vector.tensor_add(out=s_t, in0=a_t, in1=b_t)
                    nc.vector.tensor_add(out=s_t, in0=s_t, in1=c_t)
                    st = lnst.tile([128, 2, 6], F32, tag="st")
                    for s in range(2):
                        nc.vector.bn_stats(out=st[:, s, :], in_=s_t[:, s * 512:(s + 1) * 512])
                    mv = lnst.tile([128, 2], F32, tag="mv")
                    nc.vector.bn_aggr(out=mv, in_=st)
                    rstd = lnst.tile([128, 1], F32, tag="rstd")
                    nc.scalar.activation(out=rstd, in_=mv[:, 1:2],
                                         func=mybir.ActivationFunctionType.Sqrt,
                                         bias=eps_t)
                    nc.vector.reciprocal(out=rstd, in_=rstd)
                    xw = lnw.tile([128, D], F32, tag="xw")
                    nc.vector.tensor_scalar(out=xw, in0=s_t,
                                            scalar1=mv[:, 0:1], scalar2=rstd,
                                            op0=mybir.AluOpType.subtract,
                                            op1=mybir.AluOpType.mult)
                    nc.vector.tensor_mul(out=xw, in0=xw, in1=ln2s_b)
                    xn_t = lnw.tile([128, D], BF16, tag="xn_t")
                    nc.vector.tensor_add(out=xn_t, in0=xw, in1=ln2b_b)
                    for k in range(KD):
                        pt = ptr.tile([128, 128], BF16, tag="pt")
                        nc.tensor.transpose(pt, xn_t[:, k * 128:(k + 1) * 128], ident)
                        if k % 2 == 0:
                            nc.vector.tensor_copy(out=xn2T[k][:, ts_], in_=pt)
                        else:
                            nc.scalar.copy(out=xn2T[k][:, ts_], in_=pt)

            # wd pool opens after phase-1 scratch frees; DMAs overlap phase 2
            with tc.tile_pool(name="wdp", bufs=1) as wdp:
                wd_t = [wdp.tile([128, D], BF16, tag=f"wd{f}", name=f"wd{f}")
                        for f in range(NF)]
                for f in range(NF):
                    nc.sync.dma_start(out=wd_t[f], in_=wd[f])

                # ---- Phase 2: gate/up + gelu, per 128-wide f-tile ----
                with (
                    tc.tile_pool(name="wgup", bufs=3) as wgup,
                    tc.tile_pool(name="pg", bufs=2, space="PSUM") as pgp,
                    tc.tile_pool(name="pu", bufs=2, space="PSUM") as pup,
                ):
                    for f in range(NF):
                        wgu_t = wgup.tile([128, 2, KD, 128], BF16, tag="wgu")
                        nc.sync.dma_start(out=wgu_t, in_=wgu[f])
                        for c in range(2):
                            cs = slice(c * 512, (c + 1) * 512)
                            pgt = pgp.tile([128, 512], F32, tag="pg")
                            for k in range(KD):
                                nc.tensor.matmul(pgt, wgu_t[:, 0, k, :], xn2T[k][:, cs],
                                                 start=(k == 0), stop=(k == KD - 1))
                            put = pup.tile([128, 512], F32, tag="pu")
                            for k in range(KD):
                                nc.tensor.matmul(put, wgu_t[:, 1, k, :], xn2T[k][:, cs],
                                                 start=(k == 0), stop=(k == KD - 1))
                            nc.scalar.activation(out=m[f][:, cs], in_=pgt,
                                                 func=mybir.ActivationFunctionType.Gelu_apprx_tanh)
                            nc.vector.tensor_mul(out=m[f][:, cs], in0=m[f][:, cs], in1=put)

                # ---- Phase 3: down projection (token-major) + residual ----
                with (
                    tc.tile_pool(name="ow", bufs=2) as ow,
                    tc.tile_pool(name="rw", bufs=2) as rw,
                    tc.tile_pool(name="pd", bufs=2, space="PSUM") as pdp,
                ):
                    for tt in range(NTC):
                        ts_ = slice(tt * 128, (tt + 1) * 128)
                        pd = pdp.tile([128, D], F32, tag="pd")
                        for f in range(NF):
                            for c2 in range(2):
                                nc.tensor.matmul(pd[:, c2 * 512:(c2 + 1) * 512],
                                                 m[f][:, ts_],
                                                 wd_t[f][:, c2 * 512:(c2 + 1) * 512],
                                                 start=(f == 0), stop=(f == NF - 1))
                        a_t = rw.tile([128, D], BF16, tag="ra")
                        nc.sync.dma_start(out=a_t, in_=ya[ts_, :])
                        b_t = rw.tile([128, D], BF16, tag="rb")
                        nc.sync.dma_start(out=b_t, in_=yb[ts_, :])
                        c_t = rw.tile([128, D], F32, tag="rc")
                        nc.sync.dma_start(out=c_t, in_=xc[ts_, :])
                        s_t = rw.tile([128, D], F32, tag="rs")
                        nc.vector.tensor_add(out=s_t, in0=a_t, in1=b_t)
                        nc.vector.tensor_add(out=s_t, in0=s_t, in1=c_t)
                        o_t = ow.tile([128, D], F32, tag="o_t")
                        nc.vector.tensor_mul(out=o_t, in0=pd, in1=gmlp_b)
                        nc.vector.tensor_add(out=o_t, in0=o_t, in1=s_t)
                        nc.sync.dma_start(out=out[ts_, :], in_=o_t)

    nc.compile()
    return nc


# --------------------------------------------------------------------------
# Host orchestration
# --------------------------------------------------------------------------
def _pack_w_tile(w):
    """[1024, 128] (k p) c -> [128, 8, 128] p k c."""
    return np.ascontiguousarray(w.reshape(KD, 128, 128).transpose(1, 0, 2))


def prep_l1_inputs(x, cos, sin, ln1_scale, ln1_bias, w_qkv, w_o, gamma_attn):
    cosT = np.ascontiguousarray(cos.T)          # [32, 2048]
    sinT = np.ascontiguousarray(sin.T)
    cos_rep = np.tile(cosT, (4, 1)).astype(NPBF)                 # [128, 2048]
    sin_srep = np.concatenate([-sinT, sinT, -sinT, sinT], 0).astype(NPBF)
    triu = np.triu(np.ones((128, 128), np.float32)).astype(NPBF)
    wq, wk, wv = w_qkv[:, :D], w_qkv[:, D:2 * D], w_qkv[:, 2 * D:]
    f32 = lambda a: np.ascontiguousarray(np.asarray(a, np.float32))
    maps = []
    for core in range(8):
        b, g = core // 2, core % 2
        wqkv_c = np.empty((4, 3, 128, KD, 128), NPBF)
        for hp in range(4):
            cols = slice(g * 512 + hp * 128, g * 512 + (hp + 1) * 128)
            wqkv_c[hp, 0] = _pack_w_tile(wq[:, cols].astype(NPBF))
            wqkv_c[hp, 1] = _pack_w_tile(wk[:, cols].astype(NPBF))
            wqkv_c[hp, 2] = _pack_w_tile(wv[:, cols].astype(NPBF))
        wo_c = np.ascontiguousarray(
            w_o[g * 512:(g + 1) * 512, :].reshape(4, 128, D).transpose(1, 0, 2)
        ).astype(NPBF)
        maps.append({
            "x": f32(x[b]),
            "wqkv": wqkv_c,
            "wo": wo_c,
            "cos_rep": cos_rep, "sin_srep": sin_srep,
            "ln1s": f32(ln1_scale), "ln1b": f32(ln1_bias), "gattn": f32(gamma_attn),
            "triu": triu,
        })
    return maps


def prep_l2_inputs(x, y_cores, ln2_scale, ln2_bias, w_gate, w_up, w_down, gamma_mlp):
    NF = F // 128
    wgu = np.empty((NF, 128, 2, KD, 128), NPBF)
    for f in range(NF):
        fs = slice(f * 128, (f + 1) * 128)
        wgu[f, :, 0] = _pack_w_tile(np.asarray(w_gate[:, fs], np.float32).astype(NPBF))
        wgu[f, :, 1] = _pack_w_tile(np.asarray(w_up[:, fs], np.float32).astype(NPBF))
    wd_p = np.ascontiguousarray(np.asarray(w_down, np.float32).astype(NPBF)
                                .reshape(NF, 128, D))
    f32 = lambda a: np.ascontiguousarray(np.asarray(a, np.float32))
    maps = []
    for core in range(8):
        b, half = core // 2, core % 2
        rs = slice(half * 1024, (half + 1) * 1024)
        maps.append({
            "xc": f32(x[b][rs]),
            "ya": np.ascontiguousarray(y_cores[2 * b][rs]),
            "yb": np.ascontiguousarray(y_cores[2 * b + 1][rs]),
            "ln2s": f32(ln2_scale), "ln2b": f32(ln2_bias), "gmlp": f32(gamma_mlp),
            "wgu": wgu, "wd": wd_p,
        })
    return maps


_NC_CACHE = {}


def run(x, cos, sin, ln1_scale, ln1_bias, w_qkv, w_o, gamma_attn,
        ln2_scale, ln2_bias, w_gate, w_up, w_down, gamma_mlp,
        trace=False):
    f32 = lambda a: np.asarray(a, np.float32)
    x = f32(x)
    if "l1" not in _NC_CACHE:
        _NC_CACHE["l1"] = build_l1()
    if "l2" not in _NC_CACHE:
        _NC_CACHE["l2"] = build_l2()
    m1 = prep_l1_inputs(x, f32(cos), f32(sin), f32(ln1_scale), f32(ln1_bias),
                        f32(w_qkv), f32(w_o), f32(gamma_attn))
    r1 = run_bass_kernel_spmd(_NC_CACHE["l1"], m1, core_ids=list(range(8)), trace=trace)
    y_cores = [r1.results[i]["y"] for i in range(8)]
    m2 = prep_l2_inputs(x, y_cores, f32(ln2_scale), f32(ln2_bias),
                        f32(w_gate), f32(w_up), f32(w_down), f32(gamma_mlp))
    r2 = run_bass_kernel_spmd(_NC_CACHE["l2"], m2, core_ids=list(range(8)), trace=trace)
    out = np.empty((B, T, D), np.float32)
    for core in range(8):
        b, half = core // 2, core % 2
        out[b, half * 1024:(half + 1) * 1024] = r2.results[core]["out"]
    return out, (r1, r2)


def kernel(x, cos, sin, ln1_scale, ln1_bias, w_qkv, w_o, gamma_attn,
           ln2_scale, ln2_bias, w_gate, w_up, w_down, gamma_mlp):
    """Full-input / full-output entry point. Shards across 8 NeuronCores."""
    out, _ = run(x, cos, sin, ln1_scale, ln1_bias, w_qkv, w_o, gamma_attn,
                 ln2_scale, ln2_bias, w_gate, w_up, w_down, gamma_mlp)
    return out


# revision 7
# speedup vs baseline: 3.0288x; 1.4625x over previous
"""Causal transformer layer (B=4,T=2048,D=1024,F=4096,H=16) on 8 trn2 NeuronCores.

L1 (attention): core = (batch b, head-group g of 8 heads).
L2 (MLP):       core = contiguous 1024-token chunk of the flattened (B,T).

All matmul operands bf16 (1 cycle/row on TRN2 PE vs 4 for fp32), fp32 PSUM
accumulation. Attention computes scores TRANSPOSED (sT[k,q]) so exp output
feeds the A@V matmul directly as the moving operand — no per-tile PE
transposes of the attention matrix, and the softmax denominator falls out of
a ones-column folded into V. Normalization is a rank-1 broadcast matmul.
"""

import sys

sys.path.insert(0, "/opt/trn_rl_repo")

import numpy as np
import ml_dtypes

import concourse.bass as bass
import concourse.tile as tile
from concourse import bacc, mybir
from concourse.bass_utils import run_bass_kernel_spmd
from concourse.masks import make_identity

F32 = mybir.dt.float32
BF16 = mybir.dt.bfloat16
NPBF = ml_dtypes.bfloat16

B, T, D, F, H, HD = 4, 2048, 1024, 4096, 16, 64
EPS = 1e-6
NT = T // 128          # 16 token tiles (L1)
KD = D // 128          # 8 k-tiles over D
SCALE = HD ** -0.5     # 0.125, folded into exp()


def _bcast(ap, p=128):
    """Broadcast a [N] DRAM vector across p partitions -> [p, N] DMA source."""
    return bass.AP(tensor=ap.tensor, offset=ap.offset, ap=[[0, p], *list(ap.ap)])


# --------------------------------------------------------------------------
# L1: attention.  Inputs (per core):
#   x        [2048, 1024] f32   token-major batch slice
#   wqkv     [4, 3, 128, 8, 128] bf16  per-hp packed q/k/v weight tiles
#   wo       [128, 4, 1024] bf16
#   cos_rep  [128, 2048] bf16   cos[t, :32].T replicated to 4 row-blocks of 32
#   sin_srep [128, 2048] bf16   sin.T with signs [-,+,-,+] per 32-row block
#   ln1s, ln1b, gattn [1024] f32
#   triu     [128, 128] bf16    upper-triangular ones (k<=q mask in sT layout)
# Output: y [2048, 1024] bf16 = (attn @ wo) * gamma_attn  (token-major)
# --------------------------------------------------------------------------
def build_l1():
    nc = bacc.Bacc("TRN2", target_bir_lowering=False, debug=False, num_devices=8)
    x = nc.dram_tensor("x", [T, D], BF16, kind="ExternalInput").ap()
    wqkv = nc.dram_tensor("wqkv", [4, 3, 128, KD, 128], BF16, kind="ExternalInput").ap()
    wo = nc.dram_tensor("wo", [128, 4, D], BF16, kind="ExternalInput").ap()
    cos_rep = nc.dram_tensor("cos_rep", [128, T], BF16, kind="ExternalInput").ap()
    sin_srep = nc.dram_tensor("sin_srep", [128, T], BF16, kind="ExternalInput").ap()
    ln1s = nc.dram_tensor("ln1s", [D], F32, kind="ExternalInput").ap()
    ln1b = nc.dram_tensor("ln1b", [D], F32, kind="ExternalInput").ap()
    gattn = nc.dram_tensor("gattn", [D], F32, kind="ExternalInput").ap()
    triu = nc.dram_tensor("triu", [128, 128], BF16, kind="ExternalInput").ap()
    y = nc.dram_tensor("y", [T, D], BF16, kind="ExternalOutput").ap()

    with tile.TileContext(nc) as tc:
        with (
            tc.tile_pool(name="singles", bufs=1) as singles,
            tc.tile_pool(name="xnTp", bufs=1) as xnTp,
            tc.tile_pool(name="attTp", bufs=1) as attTp,
            tc.tile_pool(name="vtp", bufs=1) as vtp,
            tc.tile_pool(name="wp", bufs=1) as wp,
        ):
            ident = singles.tile([128, 128], BF16)
            make_identity(nc, ident)
            eps_t = singles.tile([128, 1], F32)
            nc.vector.memset(eps_t, EPS)
            triu_t = singles.tile([128, 128], BF16)
            nc.scalar.dma_start(out=triu_t, in_=triu)
            cos_t = singles.tile([128, T], BF16)
            nc.scalar.dma_start(out=cos_t, in_=cos_rep)
            sin_t = singles.tile([128, T], BF16)
            nc.scalar.dma_start(out=sin_t, in_=sin_srep)
            ln1s_b = singles.tile([128, D], F32)
            nc.scalar.dma_start(out=ln1s_b, in_=_bcast(ln1s))
            ln1b_b = singles.tile([128, D], F32)
            nc.scalar.dma_start(out=ln1b_b, in_=_bcast(ln1b))
            gattn_b = singles.tile([128, D], F32)
            nc.scalar.dma_start(out=gattn_b, in_=_bcast(gattn))
            ones64 = singles.tile([1, 64], BF16)
            nc.vector.memset(ones64, 1.0)

            # all projection weights up front (DMA overlaps phase A)
            wq_t = [wp.tile([128, KD, 128], BF16, tag=f"wq{hp}", name=f"wq{hp}")
                    for hp in range(4)]
            wk_t = [wp.tile([128, KD, 128], BF16, tag=f"wk{hp}", name=f"wk{hp}")
                    for hp in range(4)]
            wv_t = [wp.tile([128, KD, 128], BF16, tag=f"wv{hp}", name=f"wv{hp}")
                    for hp in range(4)]
            for hp in range(4):
                nc.scalar.dma_start(out=wq_t[hp], in_=wqkv[hp, 0])
                nc.sync.dma_start(out=wk_t[hp], in_=wqkv[hp, 1])
                nc.scalar.dma_start(out=wv_t[hp], in_=wqkv[hp, 2])
            wo_t = singles.tile([128, 4, D], BF16)
            nc.scalar.dma_start(out=wo_t, in_=wo)

            xnT = [xnTp.tile([128, T], BF16, tag=f"xnT{k}", name=f"xnT{k}")
                   for k in range(KD)]
            attT = [attTp.tile([128, T], BF16, tag=f"attT{p}", name=f"attT{p}")
                    for p in range(4)]
            # v per (hp, token-tile): [128 ktok, 65+65] = [v_h0 | 1 | v_h1 | 1]
            vt = [[vtp.tile([128, 130], BF16, tag=f"vt{hp}_{t}", name=f"vt{hp}_{t}")
                   for t in range(NT)] for hp in range(4)]
            for hp in range(4):
                for t in range(NT):
                    nc.vector.memset(vt[hp][t][:, 64:65], 1.0)
                    nc.vector.memset(vt[hp][t][:, 129:130], 1.0)

            # ---- Phase A: LN1 + transpose to feature-major + V projection ----
            with (
                tc.tile_pool(name="lnw", bufs=3) as lnw,
                tc.tile_pool(name="lnst", bufs=4) as lnst,
                tc.tile_pool(name="ptr", bufs=4, space="PSUM") as ptr,
                tc.tile_pool(name="pvp", bufs=2, space="PSUM") as pvp,
            ):
                for tt in range(NT):
                    ts_ = slice(tt * 128, (tt + 1) * 128)
                    x_t = lnw.tile([128, D], BF16, tag="x_t")
                    (nc.sync if tt % 2 == 0 else nc.scalar).dma_start(out=x_t, in_=x[ts_, :])
                    st = lnst.tile([128, 2, 6], F32, tag="st")
                    for s in range(2):
                        nc.vector.bn_stats(out=st[:, s, :], in_=x_t[:, s * 512:(s + 1) * 512])
                    mv = lnst.tile([128, 2], F32, tag="mv")
                    nc.vector.bn_aggr(out=mv, in_=st)
                    rstd = lnst.tile([128, 1], F32, tag="rstd")
                    nc.scalar.activation(out=rstd, in_=mv[:, 1:2],
                                         func=mybir.ActivationFunctionType.Sqrt,
                                         bias=eps_t)
                    nc.vector.reciprocal(out=rstd, in_=rstd)
                    xw = lnw.tile([128, D], F32, tag="xw")
                    nc.vector.tensor_scalar(out=xw, in0=x_t,
                                            scalar1=mv[:, 0:1], scalar2=rstd,
                                            op0=mybir.AluOpType.subtract,
                                            op1=mybir.AluOpType.mult)
                    nc.vector.tensor_mul(out=xw, in0=xw, in1=ln1s_b)
                    xn_t = lnw.tile([128, D], BF16, tag="xn_t")
                    nc.vector.tensor_add(out=xn_t, in0=xw, in1=ln1b_b)
                    for k in range(KD):
                        pt = ptr.tile([128, 128], BF16, tag="pt")
                        nc.tensor.transpose(pt, xn_t[:, k * 128:(k + 1) * 128], ident)
                        if k % 2 == 0:
                            nc.vector.tensor_copy(out=xnT[k][:, ts_], in_=pt)
                        else:
                            nc.scalar.copy(out=xnT[k][:, ts_], in_=pt)
                    for hp in range(4):
                        pv = pvp.tile([128, 128], F32, tag="pv")
                        for k in range(KD):
                            nc.tensor.matmul(pv, xnT[k][:, ts_], wv_t[hp][:, k, :],
                                             start=(k == 0), stop=(k == KD - 1))
                        nc.vector.tensor_copy(out=vt[hp][tt][:, 0:64], in_=pv[:, 0:64])
                        nc.vector.tensor_copy(out=vt[hp][tt][:, 65:129], in_=pv[:, 64:128])

            # ---- Phase B: per head-pair Q/K projection + rope + attention ----
            for hp in range(4):
                with (
                    tc.tile_pool(name=f"qk{hp}", bufs=1) as qkp,
                    tc.tile_pool(name=f"rot{hp}", bufs=2) as rotp,
                    tc.tile_pool(name=f"ew{hp}", bufs=3) as ewp,
                    tc.tile_pool(name=f"den{hp}", bufs=4) as denp,
                    tc.tile_pool(name=f"pqk{hp}", bufs=2, space="PSUM") as pqk,
                    tc.tile_pool(name=f"psc{hp}", bufs=2, space="PSUM") as psc,
                    tc.tile_pool(name=f"pat{hp}", bufs=2, space="PSUM") as pat,
                    tc.tile_pool(name=f"prb{hp}", bufs=2, space="PSUM") as prb,
                ):
                    qfin = qkp.tile([128, T], BF16, tag="qfin")
                    kfin = qkp.tile([128, T], BF16, tag="kfin")
                    for c in range(4):
                        cs = slice(c * 512, (c + 1) * 512)
                        for wt, dst in ((wq_t[hp], qfin), (wk_t[hp], kfin)):
                            pp = pqk.tile([128, 512], F32, tag="pp")
                            for k in range(KD):
                                nc.tensor.matmul(pp, wt[:, k, :], xnT[k][:, cs],
                                                 start=(k == 0), stop=(k == KD - 1))
                            nc.vector.tensor_copy(out=dst[:, cs], in_=pp)
                            # rope: rotate-half via 32-row SBUF shuffle DMA
                            rt = rotp.tile([128, 512], BF16, tag="rt")
                            for blk in range(4):
                                src = slice(blk * 32 + (32 if blk % 2 == 0 else -32),
                                            blk * 32 + (64 if blk % 2 == 0 else 0))
                                nc.sync.dma_start(out=rt[blk * 32:(blk + 1) * 32, :],
                                                  in_=dst[src, cs])
                            nc.vector.tensor_mul(out=rt, in0=rt, in1=sin_t[:, cs])
                            nc.vector.tensor_mul(out=dst[:, cs], in0=dst[:, cs],
                                                 in1=cos_t[:, cs])
                            nc.vector.tensor_add(out=dst[:, cs], in0=dst[:, cs], in1=rt)

                    for h in range(2):
                        rb = 64 * h
                        vs = slice(65 * h, 65 * h + 65)
                        for qc in range(4):
                            q0 = qc * 512
                            attp = pat.tile([65, 512], F32, tag="attp")
                            kt_max = 4 * qc + 3
                            for kt in range(kt_max + 1):
                                v0r = max(0, kt * 128 - q0)
                                w = 512 - v0r
                                ps = psc.tile([128, 512], F32, tag="ps")
                                nc.tensor.matmul(
                                    ps[:, :w],
                                    kfin[rb:rb + 64, kt * 128:(kt + 1) * 128],
                                    qfin[rb:rb + 64, q0 + v0r:q0 + 512])
                                eT = ewp.tile([128, 512], BF16, tag="eT")
                                nc.scalar.activation(out=eT[:, :w], in_=ps[:, :w],
                                                     func=mybir.ActivationFunctionType.Exp,
                                                     scale=SCALE)
                                if kt >= 4 * qc:
                                    nc.vector.tensor_mul(out=eT[:, 0:128],
                                                         in0=eT[:, 0:128], in1=triu_t)
                                nc.tensor.matmul(attp[:, v0r:512], vt[hp][kt][:, vs],
                                                 eT[:, :w],
                                                 start=(kt == 0), stop=(kt == kt_max),
                                                 skip_group_check=True)
                            recd = denp.tile([1, 512], F32, tag="recd")
                            nc.vector.reciprocal(out=recd, in_=attp[64:65, :])
                            recb = denp.tile([1, 512], BF16, tag="recb")
                            nc.vector.tensor_copy(out=recb, in_=recd)
                            rbp = prb.tile([64, 512], F32, tag="rbp")
                            nc.tensor.matmul(rbp, ones64, recb)
                            rbs = denp.tile([64, 512], BF16, tag="rbs")
                            nc.scalar.copy(out=rbs, in_=rbp)
                            nc.vector.tensor_mul(
                                out=attT[hp][rb:rb + 64, q0:q0 + 512],
                                in0=attp[0:64, :], in1=rbs)

            # ---- Phase C: O projection + gamma_attn ----
            with (
                tc.tile_pool(name="yw", bufs=3) as yw,
                tc.tile_pool(name="po", bufs=2, space="PSUM") as pop,
            ):
                for tt in range(NT):
                    ts_ = slice(tt * 128, (tt + 1) * 128)
                    y_t = yw.tile([128, D], BF16, tag="y_t")
                    for dc in range(2):
                        po = pop.tile([128, 512], F32, tag="po")
                        for hp in range(4):
                            nc.tensor.matmul(po, attT[hp][:, ts_],
                                             wo_t[:, hp, dc * 512:(dc + 1) * 512],
                                             start=(hp == 0), stop=(hp == 3))
                        nc.vector.tensor_mul(out=y_t[:, dc * 512:(dc + 1) * 512],
                                             in0=po, in1=gattn_b[:, dc * 512:(dc + 1) * 512])
                    (nc.sync if tt % 2 == 0 else nc.scalar).dma_start(out=y[ts_, :], in_=y_t)

    nc.compile()
    return nc


# --------------------------------------------------------------------------
# L2: MLP.  Inputs (per core, 1024-token chunk):
#   xc [1024, 1024] f32; ya, yb [1024, 1024] bf16; x2 = xc + ya + yb
#   ln2s, ln2b, gmlp [1024] f32
#   wgu [32, 128, 2, 8, 128] bf16 (per f-tile gate|up packed)
#   wd  [32, 128, 1024] bf16     (f-major row tiles)
# Output: out [1024, 1024] f32 = x2 + gmlp * (gelu_tanh(xn2@wg) * (xn2@wu)) @ wd
# --------------------------------------------------------------------------
def build_l2():
    nc = bacc.Bacc("TRN2", target_bir_lowering=False, debug=False, num_devices=8)
    TC = 1024
    NTC = TC // 128  # 8
    NF = F // 128    # 32
    xc = nc.dram_tensor("xc", [TC, D], BF16, kind="ExternalInput").ap()
    ya = nc.dram_tensor("ya", [TC, D], BF16, kind="ExternalInput").ap()
    yb = nc.dram_tensor("yb", [TC, D], BF16, kind="ExternalInput").ap()
    ln2s = nc.dram_tensor("ln2s", [D], F32, kind="ExternalInput").ap()
    ln2b = nc.dram_tensor("ln2b", [D], F32, kind="ExternalInput").ap()
    gmlp = nc.dram_tensor("gmlp", [D], F32, kind="ExternalInput").ap()
    wgu = nc.dram_tensor("wgu", [NF, 128, 2, KD, 128], BF16, kind="ExternalInput").ap()
    wd = nc.dram_tensor("wd", [NF, 128, D], BF16, kind="ExternalInput").ap()
    out = nc.dram_tensor("out", [TC, D], F32, kind="ExternalOutput").ap()

    with tile.TileContext(nc) as tc:
        with (
            tc.tile_pool(name="singles", bufs=1) as singles,
            tc.tile_pool(name="xnTp", bufs=1) as xnTp,
            tc.tile_pool(name="mp", bufs=1) as mpool,
        ):
            ident = singles.tile([128, 128], BF16)
            make_identity(nc, ident)
            eps_t = singles.tile([128, 1], F32)
            nc.vector.memset(eps_t, EPS)
            ln2s_b = singles.tile([128, D], F32)
            nc.scalar.dma_start(out=ln2s_b, in_=_bcast(ln2s))
            ln2b_b = singles.tile([128, D], F32)
            nc.scalar.dma_start(out=ln2b_b, in_=_bcast(ln2b))
            gmlp_b = singles.tile([128, D], F32)
            nc.scalar.dma_start(out=gmlp_b, in_=_bcast(gmlp))

            xn2T = [xnTp.tile([128, TC], BF16, tag=f"xn2T{k}", name=f"xn2T{k}")
                    for k in range(KD)]
            x2 = [xnTp.tile([128, D], BF16, tag=f"x2_{t}", name=f"x2_{t}")
                  for t in range(NTC)]
            m = [mpool.tile([128, TC], BF16, tag=f"m{f}", name=f"m{f}")
                 for f in range(NF)]

            # ---- Phase 1: residual add + LN2 + transpose ----
            with (
                tc.tile_pool(name="lnw", bufs=2) as lnw,
                tc.tile_pool(name="lnst", bufs=4) as lnst,
                tc.tile_pool(name="ptr", bufs=4, space="PSUM") as ptr,
            ):
                for tt in range(NTC):
                    ts_ = slice(tt * 128, (tt + 1) * 128)
                    a_t = lnw.tile([128, D], BF16, tag="a_t")
                    nc.sync.dma_start(out=a_t, in_=ya[ts_, :])
                    b_t = lnw.tile([128, D], BF16, tag="b_t")
                    nc.scalar.dma_start(out=b_t, in_=yb[ts_, :])
                    c_t = lnw.tile([128, D], BF16, tag="c_t")
                    nc.sync.dma_start(out=c_t, in_=xc[ts_, :])
                    s_t = lnw.tile([128, D], F32, tag="s_t")
                    nc.vector.tensor_add(out=s_t, in0=a_t, in1=b_t)
                    nc.vector.tensor_add(out=x2[tt], in0=s_t, in1=c_t)
                    st = lnst.tile([128, 2, 6], F32, tag="st")
                    for s in range(2):
                        nc.vector.bn_stats(out=st[:, s, :], in_=x2[tt][:, s * 512:(s + 1) * 512])
                    mv = lnst.tile([128, 2], F32, tag="mv")
                    nc.vector.bn_aggr(out=mv, in_=st)
                    rstd = lnst.tile([128, 1], F32, tag="rstd")
                    nc.scalar.activation(out=rstd, in_=mv[:, 1:2],
                                         func=mybir.ActivationFunctionType.Sqrt,
                                         bias=eps_t)
                    nc.vector.reciprocal(out=rstd, in_=rstd)
                    xw = lnw.tile([128, D], F32, tag="xw")
                    nc.vector.tensor_scalar(out=xw, in0=x2[tt],
                                            scalar1=mv[:, 0:1], scalar2=rstd,
                                            op0=mybir.AluOpType.subtract,
                                            op1=mybir.AluOpType.mult)
                    nc.vector.tensor_mul(out=xw, in0=xw, in1=ln2s_b)
                    xn_t = lnw.tile([128, D], BF16, tag="xn_t")
                    nc.vector.tensor_add(out=xn_t, in0=xw, in1=ln2b_b)
                    for k in range(KD):
                        pt = ptr.tile([128, 128], BF16, tag="pt")
                        nc.tensor.transpose(pt, xn_t[:, k * 128:(k + 1) * 128], ident)
                        if k % 2 == 0:
                            nc.vector.tensor_copy(out=xn2T[k][:, ts_], in_=pt)
                        else:
                            nc.scalar.copy(out=xn2T[k][:, ts_], in_=pt)

            # wd pool opens after phase-1 scratch frees; DMAs overlap phase 2
            with tc.tile_pool(name="wdp", bufs=1) as wdp:
                wd_t = [wdp.tile([128, D], BF16, tag=f"wd{f}", name=f"wd{f}")
                        for f in range(NF)]
                for f in range(NF):
                    (nc.sync if f % 2 == 0 else nc.scalar).dma_start(out=wd_t[f], in_=wd[f])

                # ---- Phase 2: gate/up + gelu, per 128-wide f-tile ----
                with (
                    tc.tile_pool(name="wgup", bufs=3) as wgup,
                    tc.tile_pool(name="pg", bufs=2, space="PSUM") as pgp,
                    tc.tile_pool(name="pu", bufs=2, space="PSUM") as pup,
                ):
                    for f in range(NF):
                        wgu_t = wgup.tile([128, 2, KD, 128], BF16, tag="wgu")
                        (nc.sync if f % 2 == 0 else nc.scalar).dma_start(out=wgu_t, in_=wgu[f])
                        for c in range(2):
                            cs = slice(c * 512, (c + 1) * 512)
                            pgt = pgp.tile([128, 512], F32, tag="pg")
                            for k in range(KD):
                                nc.tensor.matmul(pgt, wgu_t[:, 0, k, :], xn2T[k][:, cs],
                                                 start=(k == 0), stop=(k == KD - 1))
                            put = pup.tile([128, 512], F32, tag="pu")
                            for k in range(KD):
                                nc.tensor.matmul(put, wgu_t[:, 1, k, :], xn2T[k][:, cs],
                                                 start=(k == 0), stop=(k == KD - 1))
                            nc.scalar.activation(out=m[f][:, cs], in_=pgt,
                                                 func=mybir.ActivationFunctionType.Gelu_apprx_tanh)
                            nc.vector.tensor_mul(out=m[f][:, cs], in0=m[f][:, cs], in1=put)

                # ---- Phase 3: down projection (token-major) + residual ----
                with (
                    tc.tile_pool(name="ow", bufs=2) as ow,
                    tc.tile_pool(name="pd", bufs=2, space="PSUM") as pdp,
                ):
                    for tt in range(NTC):
                        ts_ = slice(tt * 128, (tt + 1) * 128)
                        pd = pdp.tile([128, D], F32, tag="pd")
                        for f in range(NF):
                            for c2 in range(2):
                                nc.tensor.matmul(pd[:, c2 * 512:(c2 + 1) * 512],
                                                 m[f][:, ts_],
                                                 wd_t[f][:, c2 * 512:(c2 + 1) * 512],
                                                 start=(f == 0), stop=(f == NF - 1))
                        o_t = ow.tile([128, D], F32, tag="o_t")
                        nc.vector.tensor_mul(out=o_t, in0=pd, in1=gmlp_b)
                        nc.vector.tensor_add(out=o_t, in0=o_t, in1=x2[tt])
                        (nc.sync if tt % 2 == 0 else nc.scalar).dma_start(out=out[ts_, :], in_=o_t)

    nc.compile()
    return nc


# --------------------------------------------------------------------------
# Host orchestration
# --------------------------------------------------------------------------
def _pack_w_tile(w):
    """[1024, 128] (k p) c -> [128, 8, 128] p k c."""
    return np.ascontiguousarray(w.reshape(KD, 128, 128).transpose(1, 0, 2))


def prep_l1_inputs(x, cos, sin, ln1_scale, ln1_bias, w_qkv, w_o, gamma_attn):
    cosT = np.ascontiguousarray(cos.T)          # [32, 2048]
    sinT = np.ascontiguousarray(sin.T)
    cos_rep = np.tile(cosT, (4, 1)).astype(NPBF)                 # [128, 2048]
    sin_srep = np.concatenate([-sinT, sinT, -sinT, sinT], 0).astype(NPBF)
    triu = np.triu(np.ones((128, 128), np.float32)).astype(NPBF)
    wq, wk, wv = w_qkv[:, :D], w_qkv[:, D:2 * D], w_qkv[:, 2 * D:]
    f32 = lambda a: np.ascontiguousarray(np.asarray(a, np.float32))
    maps = []
    for core in range(8):
        b, g = core // 2, core % 2
        wqkv_c = np.empty((4, 3, 128, KD, 128), NPBF)
        for hp in range(4):
            cols = slice(g * 512 + hp * 128, g * 512 + (hp + 1) * 128)
            wqkv_c[hp, 0] = _pack_w_tile(wq[:, cols].astype(NPBF))
            wqkv_c[hp, 1] = _pack_w_tile(wk[:, cols].astype(NPBF))
            wqkv_c[hp, 2] = _pack_w_tile(wv[:, cols].astype(NPBF))
        wo_c = np.ascontiguousarray(
            w_o[g * 512:(g + 1) * 512, :].reshape(4, 128, D).transpose(1, 0, 2)
        ).astype(NPBF)
        maps.append({
            "x": np.ascontiguousarray(x[b]).astype(NPBF),
            "wqkv": wqkv_c,
            "wo": wo_c,
            "cos_rep": cos_rep, "sin_srep": sin_srep,
            "ln1s": f32(ln1_scale), "ln1b": f32(ln1_bias), "gattn": f32(gamma_attn),
            "triu": triu,
        })
    return maps


def prep_l2_inputs(x, y_cores, ln2_scale, ln2_bias, w_gate, w_up, w_down, gamma_mlp):
    NF = F // 128
    wgu = np.empty((NF, 128, 2, KD, 128), NPBF)
    for f in range(NF):
        fs = slice(f * 128, (f + 1) * 128)
        wgu[f, :, 0] = _pack_w_tile(np.asarray(w_gate[:, fs], np.float32).astype(NPBF))
        wgu[f, :, 1] = _pack_w_tile(np.asarray(w_up[:, fs], np.float32).astype(NPBF))
    wd_p = np.ascontiguousarray(np.asarray(w_down, np.float32).astype(NPBF)
                                .reshape(NF, 128, D))
    f32 = lambda a: np.ascontiguousarray(np.asarray(a, np.float32))
    maps = []
    for core in range(8):
        b, half = core // 2, core % 2
        rs = slice(half * 1024, (half + 1) * 1024)
        maps.append({
            "xc": np.ascontiguousarray(x[b][rs]).astype(NPBF),
            "ya": np.ascontiguousarray(y_cores[2 * b][rs]),
            "yb": np.ascontiguousarray(y_cores[2 * b + 1][rs]),
            "ln2s": f32(ln2_scale), "ln2b": f32(ln2_bias), "gmlp": f32(gamma_mlp),
            "wgu": wgu, "wd": wd_p,
        })
    return maps


_NC_CACHE = {}


def run(x, cos, sin, ln1_scale, ln1_bias, w_qkv, w_o, gamma_attn,
        ln2_scale, ln2_bias, w_gate, w_up, w_down, gamma_mlp,
        trace=False):
    f32 = lambda a: np.asarray(a, np.float32)
    x = f32(x)
    if "l1" not in _NC_CACHE:
        _NC_CACHE["l1"] = build_l1()
    if "l2" not in _NC_CACHE:
        _NC_CACHE["l2"] = build_l2()
    m1 = prep_l1_inputs(x, f32(cos), f32(sin), f32(ln1_scale), f32(ln1_bias),
                        f32(w_qkv), f32(w_o), f32(gamma_attn))
    r1 = run_bass_kernel_spmd(_NC_CACHE["l1"], m1, core_ids=list(range(8)), trace=trace)
    y_cores = [r1.results[i]["y"] for i in range(8)]
    m2 = prep_l2_inputs(x, y_cores, f32(ln2_scale), f32(ln2_bias),
                        f32(w_gate), f32(w_up), f32(w_down), f32(gamma_mlp))
    r2 = run_bass_kernel_spmd(_NC_CACHE["l2"], m2, core_ids=list(range(8)), trace=trace)
    out = np.empty((B, T, D), np.float32)
    for core in range(8):
        b, half = core // 2, core % 2
        out[b, half * 1024:(half + 1) * 1024] = r2.results[core]["out"]
    return out, (r1, r2)


def kernel(x, cos, sin, ln1_scale, ln1_bias, w_qkv, w_o, gamma_attn,
           ln2_scale, ln2_bias, w_gate, w_up, w_down, gamma_mlp):
    """Full-input / full-output entry point. Shards across 8 NeuronCores."""
    out, _ = run(x, cos, sin, ln1_scale, ln1_bias, w_qkv, w_o, gamma_attn,
                 ln2_scale, ln2_bias, w_gate, w_up, w_down, gamma_mlp)
    return out
